# revision 8
# baseline (speedup 1.0000x reference)
"""MiniMax-Text-01 Lightning Attention on 8 Trainium2 NeuronCores (Bass/Tile).

Sharding: data-parallel over batch (2) x tensor-parallel over heads (16 -> 4
groups of 4 heads). Core c handles b = c//4, heads [4*(c%4), 4*(c%4)+4).
Per core: qkv projection (f32r matmuls, channel-major q/k, token-major v),
blockwise lightning-attention scan (BLK=256, 16 blocks), gate projection,
out projection of the per-core 512 o-channels -> partial y [4096, 2048] plus
partial sum-of-squares for the RMS norm. Host combines: y_b = sum_c y_c *
rsqrt(sum_c ssq_c / 2048 + eps) (norm scale factors out of the linear out
projection; norm_w is folded into w_out on the host).
"""
import numpy as np
from contextlib import ExitStack

import concourse.bass as bass
import concourse.tile as tile
from concourse import mybir, bacc
from concourse.bass_utils import run_bass_kernel_spmd
from concourse.masks import make_identity

B, N, H = 2, 4096, 2048
NH, HD = 16, 128
BLK = 256
EPS = 1e-6
NCORES = 8
HPC = 4               # heads per core
OC = HPC * HD         # 512 per-core channels
NCHUNK = N // BLK     # 16 chunks of 256 tokens
KT = H // 128         # 16 contraction tiles
F32 = mybir.dt.float32
F32R = mybir.dt.float32r
AF = mybir.ActivationFunctionType
ALU = mybir.AluOpType


def build_nc():
    nc = bacc.Bacc("TRN2", target_bir_lowering=False, debug=False,
                   num_devices=NCORES)
    x_d = nc.dram_tensor("x_sb", [128, KT, N], F32R, kind="ExternalInput")
    wq_d = nc.dram_tensor("wq_sb", [128, KT * OC], F32R, kind="ExternalInput")
    wk_d = nc.dram_tensor("wk_sb", [128, KT * OC], F32R, kind="ExternalInput")
    wv_d = nc.dram_tensor("wv_sb", [128, KT * OC], F32R, kind="ExternalInput")
    wg_d = nc.dram_tensor("wg_sb", [128, KT * OC], F32R, kind="ExternalInput")
    wo_d = nc.dram_tensor("wo_sb", [128, HPC * H], F32R, kind="ExternalInput")
    dT_d = nc.dram_tensor("decayT", [128, HPC * 2 * BLK], F32, kind="ExternalInput")
    qd_d = nc.dram_tensor("qdec", [128, HPC * BLK], F32, kind="ExternalInput")
    kdc_d = nc.dram_tensor("kdecc", [128, HPC * 2], F32, kind="ExternalInput")
    bd_d = nc.dram_tensor("bdec", [128, HPC], F32, kind="ExternalInput")
    ones_d = nc.dram_tensor("ones_sb", [128, 1], F32R, kind="ExternalInput")
    y_d = nc.dram_tensor("y", [N, H], F32, kind="ExternalOutput")
    ssq_d = nc.dram_tensor("ssq", [1, N], F32, kind="ExternalOutput")

    with tile.TileContext(nc) as tc, ExitStack() as ctx:
        constp = ctx.enter_context(tc.tile_pool(name="const", bufs=1))
        dramp = ctx.enter_context(tc.tile_pool(name="dramp", bufs=1, space="DRAM"))
        ot_dram = dramp.tile([128, HPC * N], F32)

        ident = constp.tile([128, 128], F32)
        make_identity(nc, ident[:])
        ones = constp.tile([128, 1], F32R)
        nc.sync.dma_start(ones[:], ones_d[:])
        dT = constp.tile([128, HPC * 2 * BLK], F32)
        nc.sync.dma_start(dT[:], dT_d[:])
        qd = constp.tile([128, HPC * BLK], F32)
        nc.sync.dma_start(qd[:], qd_d[:])
        kdc = constp.tile([128, HPC * 2], F32)
        nc.sync.dma_start(kdc[:], kdc_d[:])
        bdc = constp.tile([128, HPC], F32)
        nc.sync.dma_start(bdc[:], bd_d[:])

        # ---------------- Phase A: qkv projection + attention scan ----------
        with (
            tc.tile_pool(name="wA", bufs=1) as wA,
            tc.tile_pool(name="xA", bufs=2) as xp,
            tc.tile_pool(name="qT", bufs=8) as qTp,
            tc.tile_pool(name="kT", bufs=8) as kTp,
            tc.tile_pool(name="vt", bufs=4) as vtp,
            tc.tile_pool(name="qtd", bufs=2) as qtdp,
            tc.tile_pool(name="qkm", bufs=4) as qkmp,
            tc.tile_pool(name="otA", bufs=4) as otp,
            tc.tile_pool(name="ktok", bufs=4) as ktokp,
            tc.tile_pool(name="kv", bufs=12) as kvp,
            tc.tile_pool(name="projA", bufs=3, space="PSUM") as pprojp,
            tc.tile_pool(name="attnA", bufs=4, space="PSUM") as pattnp,
        ):
            wq = wA.tile([128, KT * OC], F32R)
            nc.sync.dma_start(wq[:], wq_d[:])
            wk = wA.tile([128, KT * OC], F32R)
            nc.sync.dma_start(wk[:], wk_d[:])
            wv = wA.tile([128, KT * OC], F32R)
            nc.sync.dma_start(wv[:], wv_d[:])

            kv_cur = [None] * HPC

            for c in range(NCHUNK):
                t0 = c * BLK
                xt = xp.tile([128, KT * BLK], F32R, tag="xt")
                nc.sync.dma_start(
                    xt[:].rearrange("p (kt t) -> p kt t", kt=KT),
                    x_d[:, :, t0:t0 + BLK],
                )

                qT, kT = [], []
                for (wmat, dstl, pool, tg) in ((wq, qT, qTp, "qT"),
                                               (wk, kT, kTp, "kT")):
                    for h in range(HPC):
                        pq = pprojp.tile([128, BLK], F32, tag="proj")
                        for kt in range(KT):
                            nc.tensor.matmul(
                                pq[:],
                                lhsT=wmat[:, kt * OC + h * HD: kt * OC + (h + 1) * HD],
                                rhs=xt[:, kt * BLK:(kt + 1) * BLK],
                                start=(kt == 0), stop=(kt == KT - 1),
                            )
                        s = pool.tile([128, BLK], F32R, tag=tg)
                        nc.scalar.activation(s[:], pq[:], AF.Silu)
                        dstl.append(s)

                vt = []
                for st in range(2):
                    pv = pprojp.tile([128, OC], F32, tag="proj")
                    for kt in range(KT):
                        nc.tensor.matmul(
                            pv[:],
                            lhsT=xt[:, kt * BLK + st * 128: kt * BLK + st * 128 + 128],
                            rhs=wv[:, kt * OC:(kt + 1) * OC],
                            start=(kt == 0), stop=(kt == KT - 1),
                        )
                    s = vtp.tile([128, OC], F32R, tag="vt")
                    nc.scalar.activation(s[:], pv[:], AF.Silu)
                    vt.append(s)

                for h in range(HPC):
                    if c > 0:
                        qTd = qtdp.tile([128, BLK], F32R, tag="qtd")
                        nc.vector.tensor_mul(qTd[:], qT[h][:], qd[:, h * BLK:(h + 1) * BLK])

                    ktok = []
                    for st in range(2):
                        ptr = pattnp.tile([128, 128], F32, tag="attn")
                        nc.tensor.transpose(ptr[:], kT[h][:, st * 128:st * 128 + 128].bitcast(F32), ident[:])
                        kt_t = ktokp.tile([128, 128], F32R, tag="ktok")
                        nc.vector.tensor_scalar_mul(
                            kt_t[:], ptr[:], kdc[:, h * 2 + st: h * 2 + st + 1])
                        ktok.append(kt_t)

                    qkm = []
                    for st in range(2):
                        pqk = pattnp.tile([128, BLK], F32, tag="attn")
                        nc.tensor.matmul(
                            pqk[:],
                            lhsT=kT[h][:, st * 128:st * 128 + 128],
                            rhs=qT[h][:],
                            start=True, stop=True,
                        )
                        qm = qkmp.tile([128, BLK], F32R, tag="qkm")
                        nc.vector.tensor_mul(
                            qm[:], pqk[:],
                            dT[:, (h * 2 + st) * BLK:(h * 2 + st + 1) * BLK])
                        qkm.append(qm)

                    po = pattnp.tile([128, BLK], F32, tag="attn")
                    nc.tensor.matmul(po[:], lhsT=vt[0][:, h * HD:(h + 1) * HD],
                                     rhs=qkm[0][:], start=True, stop=False)
                    nc.tensor.matmul(po[:], lhsT=vt[1][:, h * HD:(h + 1) * HD],
                                     rhs=qkm[1][:], start=False, stop=(c == 0))
                    if c > 0:
                        nc.tensor.matmul(po[:], lhsT=kv_cur[h][:], rhs=qTd[:],
                                         start=False, stop=True)
                    oT = otp.tile([128, BLK], F32, tag="otA")
                    nc.vector.tensor_copy(oT[:], po[:])
                    nc.sync.dma_start(ot_dram[:, h * N + t0: h * N + t0 + BLK], oT[:])

                    pkv = pattnp.tile([128, HD], F32, tag="attn")
                    nc.tensor.matmul(pkv[:], lhsT=ktok[0][:],
                                     rhs=vt[0][:, h * HD:(h + 1) * HD], start=True, stop=False)
                    nc.tensor.matmul(pkv[:], lhsT=ktok[1][:],
                                     rhs=vt[1][:, h * HD:(h + 1) * HD], start=False, stop=True)
                    kv_new = kvp.tile([128, HD], F32R, tag="kv")
                    if c == 0:
                        nc.vector.tensor_copy(kv_new[:], pkv[:])
                    else:
                        nc.vector.scalar_tensor_tensor(
                            out=kv_new[:], in0=kv_cur[h][:], scalar=bdc[:, h:h + 1],
                            in1=pkv[:], op0=ALU.mult, op1=ALU.add)
                    kv_cur[h] = kv_new

        tc.strict_bb_all_engine_barrier()

        # ---------------- Phase B: gate + rmsnorm-partial + out projection --
        with (
            tc.tile_pool(name="wB", bufs=1) as wB,
            tc.tile_pool(name="xB", bufs=2) as xpB,
            tc.tile_pool(name="gT", bufs=8) as gTp,
            tc.tile_pool(name="otB", bufs=8) as otBp,
            tc.tile_pool(name="zT", bufs=8) as zTp,
            tc.tile_pool(name="sq", bufs=2) as sqp,
            tc.tile_pool(name="ssqs", bufs=2) as ssqp,
            tc.tile_pool(name="ys", bufs=4) as ysp,
            tc.tile_pool(name="gB", bufs=2, space="PSUM") as pgp,
            tc.tile_pool(name="yB", bufs=3, space="PSUM") as pyp,
            tc.tile_pool(name="ssqB", bufs=2, space="PSUM") as pssqp,
        ):
            wg = wB.tile([128, KT * OC], F32R)
            nc.sync.dma_start(wg[:], wg_d[:])
            wo = wB.tile([128, HPC * H], F32R)
            nc.sync.dma_start(wo[:], wo_d[:])

            for c in range(NCHUNK):
                t0 = c * BLK
                xt = xpB.tile([128, KT * BLK], F32R, tag="xt")
                nc.sync.dma_start(
                    xt[:].rearrange("p (kt t) -> p kt t", kt=KT),
                    x_d[:, :, t0:t0 + BLK],
                )

                zT = []
                pssq = pssqp.tile([1, BLK], F32, tag="ssq")
                for h in range(HPC):
                    pg = pgp.tile([128, BLK], F32, tag="pg")
                    for kt in range(KT):
                        nc.tensor.matmul(
                            pg[:],
                            lhsT=wg[:, kt * OC + h * HD: kt * OC + (h + 1) * HD],
                            rhs=xt[:, kt * BLK:(kt + 1) * BLK],
                            start=(kt == 0), stop=(kt == KT - 1),
                        )
                    gT = gTp.tile([128, BLK], F32, tag="gT")
                    nc.scalar.activation(gT[:], pg[:], AF.Sigmoid)

                    oT = otBp.tile([128, BLK], F32, tag="otB")
                    nc.sync.dma_start(oT[:], ot_dram[:, h * N + t0: h * N + t0 + BLK])

                    z = zTp.tile([128, BLK], F32R, tag="zT")
                    nc.vector.tensor_mul(z[:], gT[:], oT[:])
                    zT.append(z)

                    sq = sqp.tile([128, BLK], F32R, tag="sq")
                    nc.vector.tensor_mul(sq[:], oT[:], oT[:])
                    nc.tensor.matmul(pssq[:], lhsT=ones[:], rhs=sq[:],
                                     start=(h == 0), stop=(h == HPC - 1))

                ssq_s = ssqp.tile([1, BLK], F32, tag="ssqs")
                nc.vector.tensor_copy(ssq_s[:], pssq[:])
                nc.sync.dma_start(ssq_d[0:1, t0:t0 + BLK], ssq_s[:])

                for st in range(2):
                    for fb in range(4):
                        py = pyp.tile([128, 512], F32, tag="py")
                        for h in range(HPC):
                            nc.tensor.matmul(
                                py[:],
                                lhsT=zT[h][:, st * 128:st * 128 + 128],
                                rhs=wo[:, h * H + fb * 512: h * H + (fb + 1) * 512],
                                start=(h == 0), stop=(h == HPC - 1),
                            )
                        y_s = ysp.tile([128, 512], F32, tag="ys")
                        nc.vector.tensor_copy(y_s[:], py[:])
                        nc.sync.dma_start(
                            y_d[t0 + st * 128: t0 + st * 128 + 128,
                                fb * 512:(fb + 1) * 512],
                            y_s[:],
                        )

    nc.finalize()
    return nc


_NC_CACHE = {}


def get_nc():
    if "nc" not in _NC_CACHE:
        _NC_CACHE["nc"] = build_nc()
    return _NC_CACHE["nc"]


def _prep_core_inputs(hidden_states, slope_rate, w_qkv, w_gate, w_out, norm_w):
    """Returns list of 8 in_map dicts."""
    x = np.asarray(hidden_states, dtype=np.float32)
    slopes = np.asarray(slope_rate, dtype=np.float32).reshape(NH)
    w_qkv = np.asarray(w_qkv, dtype=np.float32)
    w_gate = np.asarray(w_gate, dtype=np.float32)
    w_out = np.asarray(w_out, dtype=np.float32)
    norm_w = np.asarray(norm_w, dtype=np.float32)

    def to_sb(wT):  # [2048, F] -> [128, KT, F] -> [128, KT*F]
        f = wT.shape[1]
        return np.ascontiguousarray(
            wT.reshape(KT, 128, f).transpose(1, 0, 2).reshape(128, KT * f))

    x_sb = []
    for b in range(B):
        xT = x[b].T  # [2048, 4096]
        x_sb.append(np.ascontiguousarray(
            xT.reshape(KT, 128, N).transpose(1, 0, 2)))

    arr = np.arange(1, BLK + 1, dtype=np.float32)
    in_maps = []
    for core in range(NCORES):
        b, hg = divmod(core, HPC)
        heads = [HPC * hg + j for j in range(HPC)]
        wq = np.concatenate([w_qkv[h * 384: h * 384 + 128] for h in heads], 0)
        wk = np.concatenate([w_qkv[h * 384 + 128: h * 384 + 256] for h in heads], 0)
        wv = np.concatenate([w_qkv[h * 384 + 256: h * 384 + 384] for h in heads], 0)
        wg = w_gate[hg * OC:(hg + 1) * OC]
        wo = w_out[:, hg * OC:(hg + 1) * OC] * norm_w[None, hg * OC:(hg + 1) * OC]
        # wo: [2048 f, 512 o] -> [128 p(o), 4 ot, 2048 f]
        wo_sb = np.ascontiguousarray(
            wo.T.reshape(HPC, 128, H).transpose(1, 0, 2).reshape(128, HPC * H))

        dT = np.zeros((128, HPC * 2 * BLK), dtype=np.float32)
        qdec = np.zeros((128, HPC * BLK), dtype=np.float32)
        kdcc = np.zeros((128, HPC * 2), dtype=np.float32)
        bdec = np.zeros((128, HPC), dtype=np.float32)
        for j, h in enumerate(heads):
            s = slopes[h]
            idx = arr[:, None] - arr[None, :]  # [m, n]
            full = np.where(idx >= 0, np.exp(-s * np.maximum(idx, 0.0)), 0.0).astype(np.float32)
            for nt in range(2):
                # decayT[n, m] block: n = nt*128 + p
                dT[:, (j * 2 + nt) * BLK:(j * 2 + nt + 1) * BLK] = \
                    full.T[nt * 128:(nt + 1) * 128, :]
                kdcc[:, j * 2 + nt] = np.exp(
                    -s * (BLK - arr[nt * 128:(nt + 1) * 128]))
            qdec[:, j * BLK:(j + 1) * BLK] = np.exp(-s * arr)[None, :]
            bdec[:, j] = np.exp(-s * np.float32(BLK))

        in_maps.append({
            "ones_sb": np.ones((128, 1), dtype=np.float32),
            "x_sb": x_sb[b],
            "wq_sb": to_sb(np.ascontiguousarray(wq.T)),
            "wk_sb": to_sb(np.ascontiguousarray(wk.T)),
            "wv_sb": to_sb(np.ascontiguousarray(wv.T)),
            "wg_sb": to_sb(np.ascontiguousarray(wg.T)),
            "wo_sb": wo_sb,
            "decayT": dT,
            "qdec": qdec,
            "kdecc": kdcc,
            "bdec": bdec,
        })
    return in_maps


def _assemble(results):
    out = np.zeros((B, N, H), dtype=np.float32)
    for b in range(B):
        ys = [results[HPC * b + g]["y"] for g in range(HPC)]
        ssqs = [results[HPC * b + g]["ssq"].reshape(N) for g in range(HPC)]
        y_sum = ys[0] + ys[1] + ys[2] + ys[3]
        ssq = ssqs[0] + ssqs[1] + ssqs[2] + ssqs[3]
        rfac = 1.0 / np.sqrt(ssq / np.float32(NH * HD) + np.float32(EPS))
        out[b] = y_sum * rfac[:, None].astype(np.float32)
    return out


def kernel(hidden_states, slope_rate, w_qkv, w_gate, w_out, norm_w):
    nc = get_nc()
    in_maps = _prep_core_inputs(hidden_states, slope_rate, w_qkv, w_gate,
                                w_out, norm_w)
    res = run_bass_kernel_spmd(nc, in_maps, core_ids=list(range(NCORES)))
    return _assemble(res.results)


# revision 14
# speedup vs baseline: 1.5857x; 1.5857x over previous
"""MiniMax-Text-01 Lightning Attention on 8 Trainium2 NeuronCores (Bass/Tile).

Sharding: data-parallel over batch (2) x tensor-parallel over heads (16 -> 4
groups of 4 heads). Core c handles b = c//4, heads [4*(c%4), 4*(c%4)+4).
Per core: qkv projection (f32r matmuls, channel-major q/k, token-major v),
blockwise lightning-attention scan (BLK=256, 16 blocks), gate projection,
out projection of the per-core 512 o-channels -> partial y [4096, 2048] plus
partial sum-of-squares for the RMS norm. Host combines: y_b = sum_c y_c *
rsqrt(sum_c ssq_c / 2048 + eps) (norm scale factors out of the linear out
projection; norm_w is folded into w_out on the host).
"""
import numpy as np
from contextlib import ExitStack

import concourse.bass as bass
import concourse.tile as tile
from concourse import mybir, bacc
from concourse.bass_utils import run_bass_kernel_spmd
from concourse.masks import make_identity

B, N, H = 2, 4096, 2048
NH, HD = 16, 128
BLK = 256
EPS = 1e-6
NCORES = 8
HPC = 4               # heads per core
OC = HPC * HD         # 512 per-core channels
NCHUNK = N // BLK     # 16 chunks of 256 tokens
KT = H // 128         # 16 contraction tiles
F32 = mybir.dt.float32
F32R = mybir.dt.float32r
AF = mybir.ActivationFunctionType
ALU = mybir.AluOpType


def build_nc():
    nc = bacc.Bacc("TRN2", target_bir_lowering=False, debug=False,
                   num_devices=NCORES)
    x_d = nc.dram_tensor("x_sb", [128, KT, N], F32R, kind="ExternalInput")
    wq_d = nc.dram_tensor("wq_sb", [128, KT * OC], F32R, kind="ExternalInput")
    wk_d = nc.dram_tensor("wk_sb", [128, KT * OC], F32R, kind="ExternalInput")
    wv_d = nc.dram_tensor("wv_sb", [128, KT * OC], F32R, kind="ExternalInput")
    wg_d = nc.dram_tensor("wg_sb", [128, KT * OC], F32R, kind="ExternalInput")
    wo_d = nc.dram_tensor("wo_sb", [128, HPC * H], F32R, kind="ExternalInput")
    dT_d = nc.dram_tensor("decayT", [128, HPC * 2 * BLK], F32, kind="ExternalInput")
    qd_d = nc.dram_tensor("qdec", [128, HPC * BLK], F32, kind="ExternalInput")
    kdc_d = nc.dram_tensor("kdecc", [128, HPC * 2], F32, kind="ExternalInput")
    bd_d = nc.dram_tensor("bdec", [128, HPC], F32, kind="ExternalInput")
    ones_d = nc.dram_tensor("ones_sb", [128, 1], F32R, kind="ExternalInput")
    y_d = nc.dram_tensor("y", [N, H], F32, kind="ExternalOutput")
    ssq_d = nc.dram_tensor("ssq", [1, N], F32, kind="ExternalOutput")

    with tile.TileContext(nc) as tc, ExitStack() as ctx:
        constp = ctx.enter_context(tc.tile_pool(name="const", bufs=1))
        dramp = ctx.enter_context(tc.tile_pool(name="dramp", bufs=1, space="DRAM"))
        ot_dram = dramp.tile([128, HPC * N], F32)

        ident = constp.tile([128, 128], F32)
        make_identity(nc, ident[:])

        # ---------------- Phase A: qkv projection + attention scan ----------
        with (
            tc.tile_pool(name="wA", bufs=1) as wA,
            tc.tile_pool(name="xA", bufs=2) as xp,
            tc.tile_pool(name="qT", bufs=8) as qTp,
            tc.tile_pool(name="kT", bufs=8) as kTp,
            tc.tile_pool(name="vt", bufs=4) as vtp,
            tc.tile_pool(name="qtd", bufs=6) as qtdp,
            tc.tile_pool(name="qkm", bufs=12) as qkmp,
            tc.tile_pool(name="otA", bufs=4) as otp,
            tc.tile_pool(name="ktok", bufs=12) as ktokp,
            tc.tile_pool(name="kv", bufs=12) as kvp,
            tc.tile_pool(name="projA", bufs=3, space="PSUM") as pprojp,
            tc.tile_pool(name="attnA", bufs=5, space="PSUM") as pattnp,
        ):
            wq = wA.tile([128, KT * OC], F32R)
            wk = wA.tile([128, KT * OC], F32R)
            wv = wA.tile([128, KT * OC], F32R)
            # first x chunk, then weight pieces in consumption order (q sweep
            # runs first, then k sweep, then v) so the PE starts ~7us in
            xt0 = xp.tile([128, KT * BLK], F32R, tag="xt")
            nc.sync.dma_start(
                xt0[:].rearrange("p (kt t) -> p kt t", kt=KT),
                x_d[:, :, 0:BLK],
            )
            for (wt, wd) in ((wq, wq_d), (wk, wk_d), (wv, wv_d)):
                for kt in range(KT):
                    nc.sync.dma_start(wt[:, kt * OC:(kt + 1) * OC],
                                      wd[:, kt * OC:(kt + 1) * OC])
            dT = constp.tile([128, HPC * 2 * BLK], F32)
            nc.sync.dma_start(dT[:], dT_d[:])
            qd = constp.tile([128, HPC * BLK], F32)
            nc.sync.dma_start(qd[:], qd_d[:])
            kdc = constp.tile([128, HPC * 2], F32)
            nc.sync.dma_start(kdc[:], kdc_d[:])
            bdc = constp.tile([128, HPC], F32)
            nc.sync.dma_start(bdc[:], bd_d[:])

            kv_cur = [None] * HPC

            for c in range(NCHUNK):
                t0 = c * BLK
                if c == 0:
                    xt = xt0
                else:
                    xt = xp.tile([128, KT * BLK], F32R, tag="xt")
                    nc.sync.dma_start(
                        xt[:].rearrange("p (kt t) -> p kt t", kt=KT),
                        x_d[:, :, t0:t0 + BLK],
                    )

                qT, kT = [], []
                for (wmat, dstl, pool, tg) in ((wq, qT, qTp, "qT"),
                                               (wk, kT, kTp, "kT")):
                    for h in range(HPC):
                        pq = pprojp.tile([128, BLK], F32, tag="proj")
                        for kt in range(KT):
                            nc.tensor.matmul(
                                pq[:],
                                lhsT=wmat[:, kt * OC + h * HD: kt * OC + (h + 1) * HD],
                                rhs=xt[:, kt * BLK:(kt + 1) * BLK],
                                start=(kt == 0), stop=(kt == KT - 1),
                            )
                        s = pool.tile([128, BLK], F32R, tag=tg)
                        nc.scalar.activation(s[:], pq[:], AF.Silu)
                        dstl.append(s)

                vt = []
                for st in range(2):
                    pv = pprojp.tile([128, OC], F32, tag="proj")
                    for kt in range(KT):
                        nc.tensor.matmul(
                            pv[:],
                            lhsT=xt[:, kt * BLK + st * 128: kt * BLK + st * 128 + 128],
                            rhs=wv[:, kt * OC:(kt + 1) * OC],
                            start=(kt == 0), stop=(kt == KT - 1),
                        )
                    s = vtp.tile([128, OC], F32R, tag="vt")
                    nc.scalar.activation(s[:], pv[:], AF.Silu)
                    vt.append(s)

                # staged across heads so PE never waits on the DVE
                # drains of its own just-issued matmuls
                qkm_all, ktok_all, qTd_all = [], [], []
                for h in range(HPC):
                    qkm = []
                    for st in range(2):
                        pqk = pattnp.tile([128, BLK], F32, tag="attn")
                        nc.tensor.matmul(
                            pqk[:],
                            lhsT=kT[h][:, st * 128:st * 128 + 128],
                            rhs=qT[h][:],
                            start=True, stop=True,
                        )
                        qm = qkmp.tile([128, BLK], F32R, tag="qkm")
                        nc.vector.tensor_mul(
                            qm[:], pqk[:],
                            dT[:, (h * 2 + st) * BLK:(h * 2 + st + 1) * BLK])
                        qkm.append(qm)
                    qkm_all.append(qkm)
                for h in range(HPC):
                    ktok = []
                    for st in range(2):
                        ptr = pattnp.tile([128, 128], F32, tag="attn")
                        nc.tensor.transpose(ptr[:], kT[h][:, st * 128:st * 128 + 128].bitcast(F32), ident[:])
                        kt_t = ktokp.tile([128, 128], F32R, tag="ktok")
                        nc.vector.tensor_scalar_mul(
                            kt_t[:], ptr[:], kdc[:, h * 2 + st: h * 2 + st + 1])
                        ktok.append(kt_t)
                    ktok_all.append(ktok)
                    if c > 0:
                        qTd = qtdp.tile([128, BLK], F32R, tag="qtd")
                        nc.vector.tensor_mul(qTd[:], qT[h][:], qd[:, h * BLK:(h + 1) * BLK])
                        qTd_all.append(qTd)
                for h in range(HPC):
                    qkm = qkm_all[h]
                    po = pattnp.tile([128, BLK], F32, tag="attn")
                    nc.tensor.matmul(po[:], lhsT=vt[0][:, h * HD:(h + 1) * HD],
                                     rhs=qkm[0][:], start=True, stop=False)
                    nc.tensor.matmul(po[:], lhsT=vt[1][:, h * HD:(h + 1) * HD],
                                     rhs=qkm[1][:], start=False, stop=(c == 0))
                    if c > 0:
                        nc.tensor.matmul(po[:], lhsT=kv_cur[h][:], rhs=qTd_all[h][:],
                                         start=False, stop=True)
                    oT = otp.tile([128, BLK], F32, tag="otA")
                    nc.vector.tensor_copy(oT[:], po[:])
                    nc.sync.dma_start(ot_dram[:, h * N + t0: h * N + t0 + BLK], oT[:])
                for h in range(HPC):
                    ktok = ktok_all[h]
                    pkv = pattnp.tile([128, HD], F32, tag="attn")
                    nc.tensor.matmul(pkv[:], lhsT=ktok[0][:],
                                     rhs=vt[0][:, h * HD:(h + 1) * HD], start=True, stop=False)
                    nc.tensor.matmul(pkv[:], lhsT=ktok[1][:],
                                     rhs=vt[1][:, h * HD:(h + 1) * HD], start=False, stop=True)
                    kv_new = kvp.tile([128, HD], F32R, tag="kv")
                    if c == 0:
                        nc.vector.tensor_copy(kv_new[:], pkv[:])
                    else:
                        nc.vector.scalar_tensor_tensor(
                            out=kv_new[:], in0=kv_cur[h][:], scalar=bdc[:, h:h + 1],
                            in1=pkv[:], op0=ALU.mult, op1=ALU.add)
                    kv_cur[h] = kv_new


        # ---------------- Phase B: gate + rmsnorm-partial + out projection --
        with (
            tc.tile_pool(name="wB", bufs=1) as wB,
            tc.tile_pool(name="xB", bufs=2) as xpB,
            tc.tile_pool(name="gT", bufs=8) as gTp,
            tc.tile_pool(name="otB", bufs=8) as otBp,
            tc.tile_pool(name="zT", bufs=8) as zTp,
            tc.tile_pool(name="sq", bufs=4) as sqp,
            tc.tile_pool(name="ssqs", bufs=12) as ssqp,
            tc.tile_pool(name="ys", bufs=4) as ysp,
            tc.tile_pool(name="gB", bufs=2, space="PSUM") as pgp,
            tc.tile_pool(name="yB", bufs=3, space="PSUM") as pyp,
            tc.tile_pool(name="ssqB", bufs=2, space="PSUM") as pssqp,
        ):
            ones = constp.tile([128, 1], F32R)
            nc.sync.dma_start(ones[:], ones_d[:])
            wg = wB.tile([128, KT * OC], F32R)
            wo = wB.tile([128, HPC * H], F32R)
            xtB0 = xpB.tile([128, KT * BLK], F32R, tag="xt")
            nc.sync.dma_start(
                xtB0[:].rearrange("p (kt t) -> p kt t", kt=KT),
                x_d[:, :, 0:BLK],
            )
            for kt in range(KT):
                nc.sync.dma_start(wg[:, kt * OC:(kt + 1) * OC],
                                  wg_d[:, kt * OC:(kt + 1) * OC])
            for i in range(16):
                nc.sync.dma_start(wo[:, i * 512:(i + 1) * 512],
                                  wo_d[:, i * 512:(i + 1) * 512])

            for c in range(NCHUNK):
                t0 = c * BLK
                if c == 0:
                    xt = xtB0
                else:
                    xt = xpB.tile([128, KT * BLK], F32R, tag="xt")
                    nc.sync.dma_start(
                        xt[:].rearrange("p (kt t) -> p kt t", kt=KT),
                        x_d[:, :, t0:t0 + BLK],
                    )

                zT = []
                pssq = pssqp.tile([1, BLK], F32, tag="ssq")
                for h in range(HPC):
                    pg = pgp.tile([128, BLK], F32, tag="pg")
                    for kt in range(KT):
                        nc.tensor.matmul(
                            pg[:],
                            lhsT=wg[:, kt * OC + h * HD: kt * OC + (h + 1) * HD],
                            rhs=xt[:, kt * BLK:(kt + 1) * BLK],
                            start=(kt == 0), stop=(kt == KT - 1),
                        )
                    gT = gTp.tile([128, BLK], F32, tag="gT")
                    nc.scalar.activation(gT[:], pg[:], AF.Sigmoid)

                    oT = otBp.tile([128, BLK], F32, tag="otB")
                    nc.sync.dma_start(oT[:], ot_dram[:, h * N + t0: h * N + t0 + BLK])

                    z = zTp.tile([128, BLK], F32R, tag="zT")
                    nc.vector.tensor_mul(z[:], gT[:], oT[:])
                    zT.append(z)

                    sq = sqp.tile([128, BLK], F32R, tag="sq")
                    nc.vector.tensor_mul(sq[:], oT[:], oT[:])
                    nc.tensor.matmul(pssq[:], lhsT=ones[:], rhs=sq[:],
                                     start=(h == 0), stop=(h == HPC - 1))

                ssq_s = ssqp.tile([1, BLK], F32, tag="ssqs")
                nc.vector.tensor_copy(ssq_s[:], pssq[:])
                nc.sync.dma_start(ssq_d[0:1, t0:t0 + BLK], ssq_s[:])

                for st in range(2):
                    for fb in range(4):
                        py = pyp.tile([128, 512], F32, tag="py")
                        for h in range(HPC):
                            nc.tensor.matmul(
                                py[:],
                                lhsT=zT[h][:, st * 128:st * 128 + 128],
                                rhs=wo[:, h * H + fb * 512: h * H + (fb + 1) * 512],
                                start=(h == 0), stop=(h == HPC - 1),
                            )
                        y_s = ysp.tile([128, 512], F32, tag="ys")
                        nc.vector.tensor_copy(y_s[:], py[:])
                        nc.sync.dma_start(
                            y_d[t0 + st * 128: t0 + st * 128 + 128,
                                fb * 512:(fb + 1) * 512],
                            y_s[:],
                        )

    nc.finalize()
    return nc


_NC_CACHE = {}


def get_nc():
    if "nc" not in _NC_CACHE:
        _NC_CACHE["nc"] = build_nc()
    return _NC_CACHE["nc"]


def _prep_core_inputs(hidden_states, slope_rate, w_qkv, w_gate, w_out, norm_w):
    """Returns list of 8 in_map dicts."""
    x = np.asarray(hidden_states, dtype=np.float32)
    slopes = np.asarray(slope_rate, dtype=np.float32).reshape(NH)
    w_qkv = np.asarray(w_qkv, dtype=np.float32)
    w_gate = np.asarray(w_gate, dtype=np.float32)
    w_out = np.asarray(w_out, dtype=np.float32)
    norm_w = np.asarray(norm_w, dtype=np.float32)

    def to_sb(wT):  # [2048, F] -> [128, KT, F] -> [128, KT*F]
        f = wT.shape[1]
        return np.ascontiguousarray(
            wT.reshape(KT, 128, f).transpose(1, 0, 2).reshape(128, KT * f))

    x_sb = []
    for b in range(B):
        xT = x[b].T  # [2048, 4096]
        x_sb.append(np.ascontiguousarray(
            xT.reshape(KT, 128, N).transpose(1, 0, 2)))

    arr = np.arange(1, BLK + 1, dtype=np.float32)
    in_maps = []
    for core in range(NCORES):
        b, hg = divmod(core, HPC)
        heads = [HPC * hg + j for j in range(HPC)]
        wq = np.concatenate([w_qkv[h * 384: h * 384 + 128] for h in heads], 0)
        wk = np.concatenate([w_qkv[h * 384 + 128: h * 384 + 256] for h in heads], 0)
        wv = np.concatenate([w_qkv[h * 384 + 256: h * 384 + 384] for h in heads], 0)
        wg = w_gate[hg * OC:(hg + 1) * OC]
        wo = w_out[:, hg * OC:(hg + 1) * OC] * norm_w[None, hg * OC:(hg + 1) * OC]
        # wo: [2048 f, 512 o] -> [128 p(o), 4 ot, 2048 f]
        wo_sb = np.ascontiguousarray(
            wo.T.reshape(HPC, 128, H).transpose(1, 0, 2).reshape(128, HPC * H))

        dT = np.zeros((128, HPC * 2 * BLK), dtype=np.float32)
        qdec = np.zeros((128, HPC * BLK), dtype=np.float32)
        kdcc = np.zeros((128, HPC * 2), dtype=np.float32)
        bdec = np.zeros((128, HPC), dtype=np.float32)
        for j, h in enumerate(heads):
            s = slopes[h]
            idx = arr[:, None] - arr[None, :]  # [m, n]
            full = np.where(idx >= 0, np.exp(-s * np.maximum(idx, 0.0)), 0.0).astype(np.float32)
            for nt in range(2):
                # decayT[n, m] block: n = nt*128 + p
                dT[:, (j * 2 + nt) * BLK:(j * 2 + nt + 1) * BLK] = \
                    full.T[nt * 128:(nt + 1) * 128, :]
                kdcc[:, j * 2 + nt] = np.exp(
                    -s * (BLK - arr[nt * 128:(nt + 1) * 128]))
            qdec[:, j * BLK:(j + 1) * BLK] = np.exp(-s * arr)[None, :]
            bdec[:, j] = np.exp(-s * np.float32(BLK))

        in_maps.append({
            "ones_sb": np.ones((128, 1), dtype=np.float32),
            "x_sb": x_sb[b],
            "wq_sb": to_sb(np.ascontiguousarray(wq.T)),
            "wk_sb": to_sb(np.ascontiguousarray(wk.T)),
            "wv_sb": to_sb(np.ascontiguousarray(wv.T)),
            "wg_sb": to_sb(np.ascontiguousarray(wg.T)),
            "wo_sb": wo_sb,
            "decayT": dT,
            "qdec": qdec,
            "kdecc": kdcc,
            "bdec": bdec,
        })
    return in_maps


def _assemble(results):
    out = np.zeros((B, N, H), dtype=np.float32)
    for b in range(B):
        ys = [results[HPC * b + g]["y"] for g in range(HPC)]
        ssqs = [results[HPC * b + g]["ssq"].reshape(N) for g in range(HPC)]
        y_sum = ys[0] + ys[1] + ys[2] + ys[3]
        ssq = ssqs[0] + ssqs[1] + ssqs[2] + ssqs[3]
        rfac = 1.0 / np.sqrt(ssq / np.float32(NH * HD) + np.float32(EPS))
        out[b] = y_sum * rfac[:, None].astype(np.float32)
    return out


def kernel(hidden_states, slope_rate, w_qkv, w_gate, w_out, norm_w):
    nc = get_nc()
    in_maps = _prep_core_inputs(hidden_states, slope_rate, w_qkv, w_gate,
                                w_out, norm_w)
    res = run_bass_kernel_spmd(nc, in_maps, core_ids=list(range(NCORES)))
    return _assemble(res.results)


# revision 15
# speedup vs baseline: 72.1419x; 45.4960x over previous
"""MiniMax-Text-01 Lightning Attention on 8 Trainium2 NeuronCores (Bass/Tile).

Sharding: data-parallel over batch (2) x tensor-parallel over heads (16 -> 4
groups of 4 heads). Core c handles b = c//4, heads [4*(c%4), 4*(c%4)+4).
Per core: qkv projection (f32r matmuls, channel-major q/k, token-major v),
blockwise lightning-attention scan (BLK=256, 16 blocks), gate projection,
out projection of the per-core 512 o-channels -> partial y [4096, 2048] plus
partial sum-of-squares for the RMS norm. Host combines: y_b = sum_c y_c *
rsqrt(sum_c ssq_c / 2048 + eps) (norm scale factors out of the linear out
projection; norm_w is folded into w_out on the host).
"""
import numpy as np
from contextlib import ExitStack

import concourse.bass as bass
import concourse.tile as tile
from concourse import mybir, bacc
from concourse.bass_utils import run_bass_kernel_spmd
from concourse.masks import make_identity

B, N, H = 2, 4096, 2048
NH, HD = 16, 128
BLK = 256
EPS = 1e-6
NCORES = 8
HPC = 4               # heads per core
OC = HPC * HD         # 512 per-core channels
NCHUNK = N // BLK     # 16 chunks of 256 tokens
KT = H // 128         # 16 contraction tiles
F32 = mybir.dt.float32
F32R = mybir.dt.float32r
AF = mybir.ActivationFunctionType
ALU = mybir.AluOpType


def build_nc(repeats=1):
    nc = bacc.Bacc("TRN2", target_bir_lowering=False, debug=False,
                   num_devices=NCORES)
    x_d = nc.dram_tensor("x_sb", [128, KT, N], F32R, kind="ExternalInput")
    wq_d = nc.dram_tensor("wq_sb", [128, KT * OC], F32R, kind="ExternalInput")
    wk_d = nc.dram_tensor("wk_sb", [128, KT * OC], F32R, kind="ExternalInput")
    wv_d = nc.dram_tensor("wv_sb", [128, KT * OC], F32R, kind="ExternalInput")
    wg_d = nc.dram_tensor("wg_sb", [128, KT * OC], F32R, kind="ExternalInput")
    wo_d = nc.dram_tensor("wo_sb", [128, HPC * H], F32R, kind="ExternalInput")
    dT_d = nc.dram_tensor("decayT", [128, HPC * 2 * BLK], F32, kind="ExternalInput")
    qd_d = nc.dram_tensor("qdec", [128, HPC * BLK], F32, kind="ExternalInput")
    kdc_d = nc.dram_tensor("kdecc", [128, HPC * 2], F32, kind="ExternalInput")
    bd_d = nc.dram_tensor("bdec", [128, HPC], F32, kind="ExternalInput")
    ones_d = nc.dram_tensor("ones_sb", [128, 1], F32R, kind="ExternalInput")
    y_d = nc.dram_tensor("y", [N, H], F32, kind="ExternalOutput")
    ssq_d = nc.dram_tensor("ssq", [1, N], F32, kind="ExternalOutput")

    with tile.TileContext(nc) as tc, ExitStack() as ctx:
        constp = ctx.enter_context(tc.tile_pool(name="const", bufs=1))
        dramp = ctx.enter_context(tc.tile_pool(name="dramp", bufs=1, space="DRAM"))
        ot_dram = dramp.tile([128, HPC * N], F32)

        ident = constp.tile([128, 128], F32)
        make_identity(nc, ident[:])

        for _rep in range(repeats):
            _phases(nc, tc, constp, ident, x_d, wq_d, wk_d, wv_d, wg_d, wo_d,
                    dT_d, qd_d, kdc_d, bd_d, ones_d, y_d, ssq_d, ot_dram)

    nc.finalize()
    return nc


def _phases(nc, tc, constp, ident, x_d, wq_d, wk_d, wv_d, wg_d, wo_d,
            dT_d, qd_d, kdc_d, bd_d, ones_d, y_d, ssq_d, ot_dram):
    if True:
        # ---------------- Phase A: qkv projection + attention scan ----------
        with (
            tc.tile_pool(name="wA", bufs=1) as wA,
            tc.tile_pool(name="xA", bufs=2) as xp,
            tc.tile_pool(name="qT", bufs=8) as qTp,
            tc.tile_pool(name="kT", bufs=8) as kTp,
            tc.tile_pool(name="vt", bufs=4) as vtp,
            tc.tile_pool(name="qtd", bufs=6) as qtdp,
            tc.tile_pool(name="qkm", bufs=12) as qkmp,
            tc.tile_pool(name="otA", bufs=4) as otp,
            tc.tile_pool(name="ktok", bufs=12) as ktokp,
            tc.tile_pool(name="kv", bufs=12) as kvp,
            tc.tile_pool(name="projA", bufs=3, space="PSUM") as pprojp,
            tc.tile_pool(name="attnA", bufs=5, space="PSUM") as pattnp,
        ):
            wq = wA.tile([128, KT * OC], F32R)
            wk = wA.tile([128, KT * OC], F32R)
            wv = wA.tile([128, KT * OC], F32R)
            # first x chunk, then weight pieces in consumption order (q sweep
            # runs first, then k sweep, then v) so the PE starts ~7us in
            xt0 = xp.tile([128, KT * BLK], F32R, tag="xt")
            nc.sync.dma_start(
                xt0[:].rearrange("p (kt t) -> p kt t", kt=KT),
                x_d[:, :, 0:BLK],
            )
            for (wt, wd) in ((wq, wq_d), (wk, wk_d), (wv, wv_d)):
                for kt in range(KT):
                    nc.sync.dma_start(wt[:, kt * OC:(kt + 1) * OC],
                                      wd[:, kt * OC:(kt + 1) * OC])
            dT = constp.tile([128, HPC * 2 * BLK], F32)
            nc.sync.dma_start(dT[:], dT_d[:])
            qd = constp.tile([128, HPC * BLK], F32)
            nc.sync.dma_start(qd[:], qd_d[:])
            kdc = constp.tile([128, HPC * 2], F32)
            nc.sync.dma_start(kdc[:], kdc_d[:])
            bdc = constp.tile([128, HPC], F32)
            nc.sync.dma_start(bdc[:], bd_d[:])

            kv_cur = [None] * HPC

            for c in range(NCHUNK):
                t0 = c * BLK
                if c == 0:
                    xt = xt0
                else:
                    xt = xp.tile([128, KT * BLK], F32R, tag="xt")
                    nc.sync.dma_start(
                        xt[:].rearrange("p (kt t) -> p kt t", kt=KT),
                        x_d[:, :, t0:t0 + BLK],
                    )

                qT, kT = [], []
                for (wmat, dstl, pool, tg) in ((wq, qT, qTp, "qT"),
                                               (wk, kT, kTp, "kT")):
                    for h in range(HPC):
                        pq = pprojp.tile([128, BLK], F32, tag="proj")
                        for kt in range(KT):
                            nc.tensor.matmul(
                                pq[:],
                                lhsT=wmat[:, kt * OC + h * HD: kt * OC + (h + 1) * HD],
                                rhs=xt[:, kt * BLK:(kt + 1) * BLK],
                                start=(kt == 0), stop=(kt == KT - 1),
                            )
                        s = pool.tile([128, BLK], F32R, tag=tg)
                        nc.scalar.activation(s[:], pq[:], AF.Silu)
                        dstl.append(s)

                vt = []
                for st in range(2):
                    pv = pprojp.tile([128, OC], F32, tag="proj")
                    for kt in range(KT):
                        nc.tensor.matmul(
                            pv[:],
                            lhsT=xt[:, kt * BLK + st * 128: kt * BLK + st * 128 + 128],
                            rhs=wv[:, kt * OC:(kt + 1) * OC],
                            start=(kt == 0), stop=(kt == KT - 1),
                        )
                    s = vtp.tile([128, OC], F32R, tag="vt")
                    nc.scalar.activation(s[:], pv[:], AF.Silu)
                    vt.append(s)

                # staged across heads so PE never waits on the DVE
                # drains of its own just-issued matmuls
                qkm_all, ktok_all, qTd_all = [], [], []
                for h in range(HPC):
                    qkm = []
                    for st in range(2):
                        pqk = pattnp.tile([128, BLK], F32, tag="attn")
                        nc.tensor.matmul(
                            pqk[:],
                            lhsT=kT[h][:, st * 128:st * 128 + 128],
                            rhs=qT[h][:],
                            start=True, stop=True,
                        )
                        qm = qkmp.tile([128, BLK], F32R, tag="qkm")
                        nc.vector.tensor_mul(
                            qm[:], pqk[:],
                            dT[:, (h * 2 + st) * BLK:(h * 2 + st + 1) * BLK])
                        qkm.append(qm)
                    qkm_all.append(qkm)
                for h in range(HPC):
                    ktok = []
                    for st in range(2):
                        ptr = pattnp.tile([128, 128], F32, tag="attn")
                        nc.tensor.transpose(ptr[:], kT[h][:, st * 128:st * 128 + 128].bitcast(F32), ident[:])
                        kt_t = ktokp.tile([128, 128], F32R, tag="ktok")
                        nc.vector.tensor_scalar_mul(
                            kt_t[:], ptr[:], kdc[:, h * 2 + st: h * 2 + st + 1])
                        ktok.append(kt_t)
                    ktok_all.append(ktok)
                    if c > 0:
                        qTd = qtdp.tile([128, BLK], F32R, tag="qtd")
                        nc.vector.tensor_mul(qTd[:], qT[h][:], qd[:, h * BLK:(h + 1) * BLK])
                        qTd_all.append(qTd)
                for h in range(HPC):
                    qkm = qkm_all[h]
                    po = pattnp.tile([128, BLK], F32, tag="attn")
                    nc.tensor.matmul(po[:], lhsT=vt[0][:, h * HD:(h + 1) * HD],
                                     rhs=qkm[0][:], start=True, stop=False)
                    nc.tensor.matmul(po[:], lhsT=vt[1][:, h * HD:(h + 1) * HD],
                                     rhs=qkm[1][:], start=False, stop=(c == 0))
                    if c > 0:
                        nc.tensor.matmul(po[:], lhsT=kv_cur[h][:], rhs=qTd_all[h][:],
                                         start=False, stop=True)
                    oT = otp.tile([128, BLK], F32, tag="otA")
                    nc.vector.tensor_copy(oT[:], po[:])
                    nc.sync.dma_start(ot_dram[:, h * N + t0: h * N + t0 + BLK], oT[:])
                for h in range(HPC):
                    ktok = ktok_all[h]
                    pkv = pattnp.tile([128, HD], F32, tag="attn")
                    nc.tensor.matmul(pkv[:], lhsT=ktok[0][:],
                                     rhs=vt[0][:, h * HD:(h + 1) * HD], start=True, stop=False)
                    nc.tensor.matmul(pkv[:], lhsT=ktok[1][:],
                                     rhs=vt[1][:, h * HD:(h + 1) * HD], start=False, stop=True)
                    kv_new = kvp.tile([128, HD], F32R, tag="kv")
                    if c == 0:
                        nc.vector.tensor_copy(kv_new[:], pkv[:])
                    else:
                        nc.vector.scalar_tensor_tensor(
                            out=kv_new[:], in0=kv_cur[h][:], scalar=bdc[:, h:h + 1],
                            in1=pkv[:], op0=ALU.mult, op1=ALU.add)
                    kv_cur[h] = kv_new


        # ---------------- Phase B: gate + rmsnorm-partial + out projection --
        with (
            tc.tile_pool(name="wB", bufs=1) as wB,
            tc.tile_pool(name="xB", bufs=2) as xpB,
            tc.tile_pool(name="gT", bufs=8) as gTp,
            tc.tile_pool(name="otB", bufs=8) as otBp,
            tc.tile_pool(name="zT", bufs=8) as zTp,
            tc.tile_pool(name="sq", bufs=4) as sqp,
            tc.tile_pool(name="ssqs", bufs=12) as ssqp,
            tc.tile_pool(name="ys", bufs=4) as ysp,
            tc.tile_pool(name="gB", bufs=2, space="PSUM") as pgp,
            tc.tile_pool(name="yB", bufs=3, space="PSUM") as pyp,
            tc.tile_pool(name="ssqB", bufs=2, space="PSUM") as pssqp,
        ):
            ones = constp.tile([128, 1], F32R)
            nc.sync.dma_start(ones[:], ones_d[:])
            wg = wB.tile([128, KT * OC], F32R)
            wo = wB.tile([128, HPC * H], F32R)
            xtB0 = xpB.tile([128, KT * BLK], F32R, tag="xt")
            nc.sync.dma_start(
                xtB0[:].rearrange("p (kt t) -> p kt t", kt=KT),
                x_d[:, :, 0:BLK],
            )
            for kt in range(KT):
                nc.sync.dma_start(wg[:, kt * OC:(kt + 1) * OC],
                                  wg_d[:, kt * OC:(kt + 1) * OC])
            for i in range(16):
                nc.sync.dma_start(wo[:, i * 512:(i + 1) * 512],
                                  wo_d[:, i * 512:(i + 1) * 512])

            for c in range(NCHUNK):
                t0 = c * BLK
                if c == 0:
                    xt = xtB0
                else:
                    xt = xpB.tile([128, KT * BLK], F32R, tag="xt")
                    nc.sync.dma_start(
                        xt[:].rearrange("p (kt t) -> p kt t", kt=KT),
                        x_d[:, :, t0:t0 + BLK],
                    )

                zT = []
                pssq = pssqp.tile([1, BLK], F32, tag="ssq")
                for h in range(HPC):
                    pg = pgp.tile([128, BLK], F32, tag="pg")
                    for kt in range(KT):
                        nc.tensor.matmul(
                            pg[:],
                            lhsT=wg[:, kt * OC + h * HD: kt * OC + (h + 1) * HD],
                            rhs=xt[:, kt * BLK:(kt + 1) * BLK],
                            start=(kt == 0), stop=(kt == KT - 1),
                        )
                    gT = gTp.tile([128, BLK], F32, tag="gT")
                    nc.scalar.activation(gT[:], pg[:], AF.Sigmoid)

                    oT = otBp.tile([128, BLK], F32, tag="otB")
                    nc.sync.dma_start(oT[:], ot_dram[:, h * N + t0: h * N + t0 + BLK])

                    z = zTp.tile([128, BLK], F32R, tag="zT")
                    nc.vector.tensor_mul(z[:], gT[:], oT[:])
                    zT.append(z)

                    sq = sqp.tile([128, BLK], F32R, tag="sq")
                    nc.vector.tensor_mul(sq[:], oT[:], oT[:])
                    nc.tensor.matmul(pssq[:], lhsT=ones[:], rhs=sq[:],
                                     start=(h == 0), stop=(h == HPC - 1))

                ssq_s = ssqp.tile([1, BLK], F32, tag="ssqs")
                nc.vector.tensor_copy(ssq_s[:], pssq[:])
                nc.sync.dma_start(ssq_d[0:1, t0:t0 + BLK], ssq_s[:])

                for st in range(2):
                    for fb in range(4):
                        py = pyp.tile([128, 512], F32, tag="py")
                        for h in range(HPC):
                            nc.tensor.matmul(
                                py[:],
                                lhsT=zT[h][:, st * 128:st * 128 + 128],
                                rhs=wo[:, h * H + fb * 512: h * H + (fb + 1) * 512],
                                start=(h == 0), stop=(h == HPC - 1),
                            )
                        y_s = ysp.tile([128, 512], F32, tag="ys")
                        nc.vector.tensor_copy(y_s[:], py[:])
                        nc.sync.dma_start(
                            y_d[t0 + st * 128: t0 + st * 128 + 128,
                                fb * 512:(fb + 1) * 512],
                            y_s[:],
                        )


_NC_CACHE = {}


def get_nc():
    if "nc" not in _NC_CACHE:
        _NC_CACHE["nc"] = build_nc()
    return _NC_CACHE["nc"]


def _prep_core_inputs(hidden_states, slope_rate, w_qkv, w_gate, w_out, norm_w):
    """Returns list of 8 in_map dicts."""
    x = np.asarray(hidden_states, dtype=np.float32)
    slopes = np.asarray(slope_rate, dtype=np.float32).reshape(NH)
    w_qkv = np.asarray(w_qkv, dtype=np.float32)
    w_gate = np.asarray(w_gate, dtype=np.float32)
    w_out = np.asarray(w_out, dtype=np.float32)
    norm_w = np.asarray(norm_w, dtype=np.float32)

    def to_sb(wT):  # [2048, F] -> [128, KT, F] -> [128, KT*F]
        f = wT.shape[1]
        return np.ascontiguousarray(
            wT.reshape(KT, 128, f).transpose(1, 0, 2).reshape(128, KT * f))

    x_sb = []
    for b in range(B):
        xT = x[b].T  # [2048, 4096]
        x_sb.append(np.ascontiguousarray(
            xT.reshape(KT, 128, N).transpose(1, 0, 2)))

    arr = np.arange(1, BLK + 1, dtype=np.float32)
    in_maps = []
    for core in range(NCORES):
        b, hg = divmod(core, HPC)
        heads = [HPC * hg + j for j in range(HPC)]
        wq = np.concatenate([w_qkv[h * 384: h * 384 + 128] for h in heads], 0)
        wk = np.concatenate([w_qkv[h * 384 + 128: h * 384 + 256] for h in heads], 0)
        wv = np.concatenate([w_qkv[h * 384 + 256: h * 384 + 384] for h in heads], 0)
        wg = w_gate[hg * OC:(hg + 1) * OC]
        wo = w_out[:, hg * OC:(hg + 1) * OC] * norm_w[None, hg * OC:(hg + 1) * OC]
        # wo: [2048 f, 512 o] -> [128 p(o), 4 ot, 2048 f]
        wo_sb = np.ascontiguousarray(
            wo.T.reshape(HPC, 128, H).transpose(1, 0, 2).reshape(128, HPC * H))

        dT = np.zeros((128, HPC * 2 * BLK), dtype=np.float32)
        qdec = np.zeros((128, HPC * BLK), dtype=np.float32)
        kdcc = np.zeros((128, HPC * 2), dtype=np.float32)
        bdec = np.zeros((128, HPC), dtype=np.float32)
        for j, h in enumerate(heads):
            s = slopes[h]
            idx = arr[:, None] - arr[None, :]  # [m, n]
            full = np.where(idx >= 0, np.exp(-s * np.maximum(idx, 0.0)), 0.0).astype(np.float32)
            for nt in range(2):
                # decayT[n, m] block: n = nt*128 + p
                dT[:, (j * 2 + nt) * BLK:(j * 2 + nt + 1) * BLK] = \
                    full.T[nt * 128:(nt + 1) * 128, :]
                kdcc[:, j * 2 + nt] = np.exp(
                    -s * (BLK - arr[nt * 128:(nt + 1) * 128]))
            qdec[:, j * BLK:(j + 1) * BLK] = np.exp(-s * arr)[None, :]
            bdec[:, j] = np.exp(-s * np.float32(BLK))

        in_maps.append({
            "ones_sb": np.ones((128, 1), dtype=np.float32),
            "x_sb": x_sb[b],
            "wq_sb": to_sb(np.ascontiguousarray(wq.T)),
            "wk_sb": to_sb(np.ascontiguousarray(wk.T)),
            "wv_sb": to_sb(np.ascontiguousarray(wv.T)),
            "wg_sb": to_sb(np.ascontiguousarray(wg.T)),
            "wo_sb": wo_sb,
            "decayT": dT,
            "qdec": qdec,
            "kdecc": kdcc,
            "bdec": bdec,
        })
    return in_maps


def _assemble(results):
    out = np.zeros((B, N, H), dtype=np.float32)
    for b in range(B):
        ys = [results[HPC * b + g]["y"] for g in range(HPC)]
        ssqs = [results[HPC * b + g]["ssq"].reshape(N) for g in range(HPC)]
        y_sum = ys[0] + ys[1] + ys[2] + ys[3]
        ssq = ssqs[0] + ssqs[1] + ssqs[2] + ssqs[3]
        rfac = 1.0 / np.sqrt(ssq / np.float32(NH * HD) + np.float32(EPS))
        out[b] = y_sum * rfac[:, None].astype(np.float32)
    return out


def kernel(hidden_states, slope_rate, w_qkv, w_gate, w_out, norm_w):
    nc = get_nc()
    in_maps = _prep_core_inputs(hidden_states, slope_rate, w_qkv, w_gate,
                                w_out, norm_w)
    res = run_bass_kernel_spmd(nc, in_maps, core_ids=list(range(NCORES)))
    return _assemble(res.results)
